# revision 14
# baseline (speedup 1.0000x reference)
"""DISCO (discrete-continuous) S2 conv encoder for Trainium2, 8-core SPMD.

Math (per output latitude h):
    z[k, c, w] = sum_n psi[k, h, n] * x[c, hi[h,n], (wi[h,n] + 2w) mod Win]
    y[o, h, w] = sum_{c,k} weight[o, c, k] * z[k, c, w]

Strategy:
  * Shard output latitudes (Hout) across the 8 cores; weight/psi tables
    replicated; each core receives the full (re-laid-out) x.
  * Host pre-lays x out as parity-split (even/odd longitude), cyclically
    padded, channel-minor rows:  xpad[par, r, w', c], w' in [0, 2*Wout-1).
    Then the (h, n) support slice {x[c, hi, wi + 2w] : w in [w0, w0+WB),
    all c} is ONE contiguous f16 block, so the device gathers 128 of them
    (one per neighbor n) with a single indirect DMA per (h, w-block).
  * matmul1 contracts n (=128, the partition dim) with the per-h psi basis
    vT [128, K]; 4 latitudes run concurrently in separate 32-column
    tile_position groups of the PE array.
  * z is evacuated from PSUM (f32->f16) and transposed via SBUF->SBUF DMA
    into a (k,c)-partition layout z' [K*CG, NG, Wout].
  * matmul2 contracts (c,k) in NG accumulating chunks against the packed
    weight w2 [K*CG, NG, Cout] to produce y[h] in PSUM, evacuated f32.
"""

import math
from contextlib import ExitStack
from dataclasses import dataclass

import numpy as np


# ---------------------------------------------------------------- dims

@dataclass(frozen=True)
class Dims:
    Cin: int = 73
    Hin: int = 721
    Win: int = 1440
    Cout: int = 256
    K: int = 9
    Hout: int = 361
    N: int = 128
    pscale: int = 2
    ncores: int = 8
    GH: int = 4     # latitudes processed concurrently (PE col-tile groups)
    WB: int = 90    # w-block size for the gather/matmul1 pipeline
    XB: int = 2     # gather buffer depth per latitude tag
    NHALF: int = 8  # z4 is staged in NHALF w-spans, transposed per span
    # "full" | "fakegather" | pipeline truncations for perf ablation:
    # "gather" (gathers+y only), "mm1", "evac", "tr" (everything but mm2)
    mode: str = "full"
    REP: int = 1    # repeat the whole body (device-side timing ablation)

    @property
    def stage(self):
        order = {"gather": 1, "mm1": 2, "evac": 3, "tr": 4,
                 "full": 5, "fakegather": 5}
        return order[self.mode]

    @property
    def Wout(self):
        return self.Win // self.pscale

    @property
    def WPAD(self):
        return 2 * self.Wout - 1

    @property
    def TOT(self):
        return 2 * self.Hin * self.WPAD * self.Cin

    @property
    def NWB(self):
        assert self.Wout % self.WB == 0
        return self.Wout // self.WB

    @property
    def WBH(self):  # w-blocks per z4 half-span
        assert self.NWB % self.NHALF == 0
        return self.NWB // self.NHALF

    @property
    def HWB(self):  # w width of a z4 span
        return self.WBH * self.WB

    @property
    def HB(self):  # padded per-core latitude slots
        per = math.ceil(self.Hout / self.ncores)
        return math.ceil(per / self.GH) * self.GH

    @property
    def NGRP(self):
        return self.HB // self.GH

    @property
    def KP(self):  # psi k-dim padded to a full PE column-tile group
        return 32

    @property
    def CG(self):  # channels per (c,k) partition group of z'
        return min(128 // self.K, self.Cin)

    @property
    def P2(self):
        return self.K * self.CG

    @property
    def NG(self):
        return math.ceil(self.Cin / self.CG)

    @property
    def CPAD(self):  # channels padded to uniform groups (pad cols are zero)
        return self.NG * self.CG

    @property
    def CC(self):  # channel chunk for matmul1 psum (<=512 f32 per bank);
        # divides CG so chunks never straddle a (g) block of z4's
        # (co, g, w) free layout
        lim = max(1, min(512 // self.WB, self.CG))
        return max(c for c in range(1, lim + 1) if self.CG % c == 0)

    @property
    def NCC(self):
        return math.ceil(self.Cin / self.CC)

    @property
    def OH(self):  # output-channel halves
        return math.ceil(self.Cout / 128)

    @property
    def OHW(self):
        return min(self.Cout, 128)

    @property
    def NWH(self):  # w-halves for matmul2 (<=512 f32 psum)
        return math.ceil(self.Wout / 512)

    @property
    def WHS(self):
        assert self.Wout % self.NWH == 0
        return self.Wout // self.NWH

    def check(self):
        assert self.N <= 128
        assert self.K <= 32
        assert 32 * (self.GH - 1) + self.K <= 128
        assert self.CC * self.WB <= 512
        assert self.WHS <= 512
        assert self.Win == 2 * self.Wout


# ---------------------------------------------------------------- device program

def build_nc(d: Dims):
    import concourse.bacc as bacc
    import concourse.bass as bass
    import concourse.tile as tile
    from concourse import mybir

    F16 = mybir.dt.float16
    F32 = mybir.dt.float32
    I32 = mybir.dt.int32

    d.check()
    nc = bacc.Bacc("TRN2", target_bir_lowering=False, debug=False,
                   num_devices=d.ncores)

    xpad = nc.declare_dram_parameter(
        "xpad", [2 * d.Hin, d.WPAD * d.Cin], F16, isOutput=False)
    gidx = nc.declare_dram_parameter("gidx", [d.N, d.HB], I32, isOutput=False)
    vt = nc.declare_dram_parameter("vt", [d.N, d.HB * d.KP], F16,
                                   isOutput=False)
    w2 = nc.declare_dram_parameter("w2", [d.P2, d.NG * d.Cout], F16,
                                   isOutput=False)
    y = nc.declare_dram_parameter("y", [d.HB, d.Cout, d.Wout], F16,
                                  isOutput=True)

    with tile.TileContext(nc) as tc, ExitStack() as ctx:
        const = ctx.enter_context(tc.tile_pool(name="const", bufs=1))
        xgp = ctx.enter_context(tc.tile_pool(name="xgp", bufs=2))
        z4p = ctx.enter_context(tc.tile_pool(name="z4p", bufs=2))
        zpsp = ctx.enter_context(tc.tile_pool(name="zpsp", bufs=4, space="PSUM"))
        zprp = ctx.enter_context(tc.tile_pool(name="zprp", bufs=1))
        ypsp = ctx.enter_context(tc.tile_pool(name="ypsp", bufs=2, space="PSUM"))
        ysbp = ctx.enter_context(tc.tile_pool(name="ysbp", bufs=3))

        gidx_sb = const.tile([d.N, d.HB], I32, name="gidx_sb")
        nc.sync.dma_start(out=gidx_sb[:], in_=gidx[:])
        vt_sb = const.tile([d.N, d.HB * d.KP], F16, name="vt_sb")
        nc.sync.dma_start(out=vt_sb[:], in_=vt[:])
        w2_sb = const.tile([d.P2, d.NG * d.Cout], F16, name="w2_sb")
        nc.sync.dma_start(out=w2_sb[:], in_=w2[:])

        vt_v = vt_sb.rearrange("n (h k) -> n h k", k=d.KP)
        w2_v = w2_sb.rearrange("p (g o) -> p g o", g=d.NG)

        # persistent z' tiles (one per concurrent latitude slot).
        # All groups are uniform (channels padded to CPAD with zeros in
        # z4's pad cols and in w2's pad rows), laid out p = k*CG + co.
        zpr = [zprp.tile([d.P2, d.NG * d.Wout], F16, name=f"zpr_{j}",
                         tag=f"zpr_{j}") for j in range(d.GH)]

        # z4 free layout is (co, g, w): channel c = g*CG + co sits at
        # free block co*NG + g, so the merged transpose's src AP is 3-dim
        # ((co,g) fuse into one stride-HWB dim in (k, co, g, w) order).
        # Pre-zero the channel-pad region (g = NG-1, co >= CREM) once —
        # tiles are tag-stable — so w2's zero pad rows see exact zeros.
        crem = d.Cin - d.CG * (d.NG - 1)
        for t in range(2):
            z4t = z4p.tile([128, d.CPAD * d.HWB], F16, name=f"z4{t}",
                           tag=f"z4{t}", bufs=1)
            nc.vector.memset(
                z4t.rearrange("p (co g w) -> p co g w",
                              co=d.CG, g=d.NG)[:, crem:, d.NG - 1, :],
                0.0)

        for grp in range(d.NGRP * d.REP):
            grp = grp % d.NGRP
            hs = [grp * d.GH + j for j in range(d.GH)]
            for wb in range(d.NWB):
                half, wl = divmod(wb, d.WBH)
                xg = []
                for j in range(d.GH):
                    xgt = xgp.tile([d.N, d.WB * d.Cin], F16, name=f"xg{j}",
                                   tag=f"xg{j}", bufs=d.XB)
                    if d.mode == "fakegather":
                        r0 = ((grp * d.NWB + wb) * d.GH + j) % \
                            (2 * d.Hin - d.N)
                        nc.sync.dma_start(
                            out=xgt[:],
                            in_=xpad[r0:r0 + d.N,
                                     wb * d.WB * d.Cin:(wb + 1) * d.WB * d.Cin])
                    else:
                        nc.gpsimd.indirect_dma_start(
                            out=xgt[:],
                            out_offset=None,
                            in_=xpad[:],
                            in_offset=bass.IndirectOffsetOnAxis(
                                ap=gidx_sb[:, hs[j]:hs[j] + 1], axis=1),
                            element_offset=wb * d.WB * d.Cin,
                        )
                    xg.append(xgt)

                if wl == 0:
                    # only 2 spans alive at once: the one being written and
                    # the previous one being transposed out
                    z4 = z4p.tile([128, d.CPAD * d.HWB], F16,
                                  name=f"z4{half % 2}",
                                  tag=f"z4{half % 2}", bufs=1)
                    z4_v = z4.rearrange("p (co g w) -> p co g w",
                                        co=d.CG, g=d.NG)
                if d.stage < 2:
                    continue
                for cc in range(d.NCC):
                    c0 = cc * d.CC
                    cw = min(d.CC, d.Cin - c0)
                    zps = zpsp.tile([128, d.CC * d.WB], F32, name="zps")
                    for j in range(d.GH):
                        rhs = xg[j].rearrange("n (w c) -> n c w",
                                              c=d.Cin)[:, c0:c0 + cw, :]
                        nc.tensor.matmul(
                            out=zps[32 * j:32 * (j + 1), :cw * d.WB],
                            lhsT=vt_v[:, hs[j], :],
                            rhs=rhs,
                            start=True, stop=True,
                            tile_position=(0, 32 * j),
                        )
                    if d.stage < 3:
                        continue
                    # alternate PSUM evacuation between DVE and ACT so the
                    # two engines each carry half the copy stream. The
                    # c-chunk [c0, c0+cw) lies inside g-block c0//CG since
                    # CC divides CG.
                    g0, co0 = divmod(c0, d.CG)
                    dst = z4_v[:32 * d.GH, co0:co0 + cw, g0,
                               wl * d.WB:(wl + 1) * d.WB]
                    src = zps.rearrange("p (c w) -> p c w",
                                        c=d.CC)[:32 * d.GH, :cw, :]
                    if cc % 2 == 0:
                        nc.vector.tensor_copy(out=dst, in_=src)
                    else:
                        nc.scalar.activation(
                            out=dst, in_=src,
                            func=mybir.ActivationFunctionType.Copy)

                if d.stage < 4 or wl != d.WBH - 1:
                    continue
                # span complete: one merged transpose per latitude slot.
                # src (k, (co g), w) walks elements in the same order as
                # dst (p=(k,co), g, w); both APs are 3-dim.
                z4_t = z4.rearrange("p (cg w) -> p cg w", w=d.HWB)
                for j in range(d.GH):
                    zv = zpr[j].rearrange("p (g w) -> p g w", g=d.NG)
                    nc.sync.dma_start(
                        out=zv[:d.P2, :, half * d.HWB:(half + 1) * d.HWB],
                        in_=z4_t[32 * j:32 * j + d.K],
                    )

            if d.stage < 5:
                # keep the y-write volume, fed from a junk tile
                for j in range(d.GH):
                    for oh in range(d.OH):
                        o0 = oh * d.OHW
                        ow = min(d.OHW, d.Cout - o0)
                        ysbt = ysbp.tile([d.OHW, d.Wout], F16, name="ysbt")
                        nc.vector.memset(ysbt[:1, :1], 0.0)
                        nc.sync.dma_start(
                            out=y[hs[j], o0:o0 + ow, :], in_=ysbt[:ow, :])
                continue
            for j in range(d.GH):
                zj = zpr[j].rearrange("p (g w) -> p g w", g=d.NG)
                for oh in range(d.OH):
                    o0 = oh * d.OHW
                    ow = min(d.OHW, d.Cout - o0)
                    ysbt = ysbp.tile([d.OHW, d.Wout], F16, name="ysbt")
                    yps = [ypsp.tile([d.OHW, d.WHS], F32, name=f"yps{wh}",
                                     tag=f"yps{wh}")
                           for wh in range(d.NWH)]
                    for g in range(d.NG):
                        for wh in range(d.NWH):
                            nc.tensor.matmul(
                                out=yps[wh][:ow, :],
                                lhsT=w2_v[:d.P2, g, o0:o0 + ow],
                                rhs=zj[:d.P2, g,
                                       wh * d.WHS:(wh + 1) * d.WHS],
                                start=(g == 0), stop=(g == d.NG - 1),
                            )
                    for wh in range(d.NWH):
                        dst = ysbt[:ow, wh * d.WHS:(wh + 1) * d.WHS]
                        if (oh + wh) % 2 == 0:
                            nc.vector.tensor_copy(out=dst, in_=yps[wh][:ow, :])
                        else:
                            nc.scalar.activation(
                                out=dst, in_=yps[wh][:ow, :],
                                func=mybir.ActivationFunctionType.Copy)
                    nc.sync.dma_start(
                        out=y[hs[j], o0:o0 + ow, :],
                        in_=ysbt[:ow, :])

    nc.finalize()
    return nc


# ---------------------------------------------------------------- host side

def prep_xpad(x, d: Dims):
    xr = np.transpose(x[0], (1, 2, 0))  # [Hin, Win, Cin]
    xs = xr.reshape(d.Hin, d.Wout, d.pscale, d.Cin).transpose(2, 0, 1, 3)
    xpad = np.empty((2, d.Hin, d.WPAD, d.Cin), dtype=np.float16)
    xpad[:, :, :d.Wout] = xs
    xpad[:, :, d.Wout:] = xs[:, :, :d.WPAD - d.Wout]
    return np.ascontiguousarray(xpad).reshape(2 * d.Hin, d.WPAD * d.Cin)


def core_h_ranges(d: Dims):
    base, rem = divmod(d.Hout, d.ncores)
    counts = [base + (1 if p < rem else 0) for p in range(d.ncores)]
    offs = np.concatenate([[0], np.cumsum(counts)])
    return [(int(offs[p]), counts[p]) for p in range(d.ncores)]


def prep_core_tables(psi_vals, idx_hi, idx_wi, d: Dims, h0, cnt):
    hg = np.minimum(h0 + np.arange(d.HB), d.Hout - 1)  # pad with a valid h
    wi = idx_wi[hg]                      # [HB, N]
    par = wi % 2
    m = wi // 2
    r = idx_hi[hg]
    flat = ((par.astype(np.int64) * d.Hin + r) * d.WPAD + m) * d.Cin
    assert flat.max() + d.Wout * d.Cin <= d.TOT
    gidx = flat.astype(np.int32).T.copy()           # [N, HB]
    vt = np.zeros((d.N, d.HB, d.KP), dtype=np.float16)
    vt[:, :, :d.K] = psi_vals[:, hg, :].transpose(2, 1, 0)
    return gidx, vt.reshape(d.N, d.HB * d.KP)


def prep_w2(weight, d: Dims):
    w = weight.transpose(1, 2, 0)  # [Cin, K, Cout]
    wp = np.zeros((d.CPAD, d.K, d.Cout), dtype=np.float16)
    wp[:d.Cin] = w
    # rows p = k*CG + co, uniform across groups (pad rows are zero)
    w2 = (wp.reshape(d.NG, d.CG, d.K, d.Cout)
            .transpose(0, 2, 1, 3)          # [NG, K, CG, Cout]
            .reshape(d.NG, d.P2, d.Cout)
            .transpose(1, 0, 2))            # [P2, NG, Cout]
    return np.ascontiguousarray(w2.reshape(d.P2, d.NG * d.Cout))


_NC_CACHE = {}


def _get_nc(d: Dims):
    if d not in _NC_CACHE:
        _NC_CACHE[d] = build_nc(d)
    return _NC_CACHE[d]


def make_in_maps(x, weight, psi_vals, idx_hi, idx_wi, d: Dims):
    xpad = prep_xpad(x, d)
    w2 = prep_w2(weight, d)
    in_maps = []
    for h0, cnt in core_h_ranges(d):
        gidx, vt = prep_core_tables(psi_vals, idx_hi, idx_wi, d, h0, cnt)
        in_maps.append({"xpad": xpad, "gidx": gidx, "vt": vt, "w2": w2})
    return in_maps


def assemble_y(per_core_y, d: Dims):
    parts = [per_core_y[p][:cnt] for p, (h0, cnt) in enumerate(core_h_ranges(d))]
    yh = np.concatenate(parts, axis=0)          # [Hout, Cout, Wout]
    return yh.transpose(1, 0, 2)[None]          # [1, Cout, Hout, Wout]


def kernel(x, weight, psi_vals, idx_hi, idx_wi):
    from concourse.bass_utils import run_bass_kernel_spmd

    d = Dims()
    nc = _get_nc(d)
    in_maps = make_in_maps(x, weight, psi_vals, idx_hi, idx_wi, d)
    res = run_bass_kernel_spmd(nc, in_maps, list(range(d.ncores)))
    ys = [res.results[p]["y"] for p in range(d.ncores)]
    return assemble_y(ys, d).astype(x.dtype)



# revision 16
# speedup vs baseline: 1.7713x; 1.7713x over previous
"""DISCO (discrete-continuous) S2 conv encoder for Trainium2, 8-core SPMD.

Math (per output latitude h):
    z[k, c, w] = sum_n psi[k, h, n] * x[c, hi[h,n], (wi[h,n] + 2w) mod Win]
    y[o, h, w] = sum_{o,c,k} weight[o, c, k] * z[k, c, w]

Strategy:
  * Shard output latitudes (Hout) across the 8 cores; weight/psi tables
    replicated; each core receives the full (re-laid-out) x.
  * Host pre-lays x out as parity-split (even/odd longitude), cyclically
    padded, channel-minor rows:  xpad[par, r, w', c], w' in [0, 2*Wout-1).
    Then the (h, n) support slice {x[c, hi, wi + 2w] : w in [w0, w0+WB),
    all c} is ONE contiguous f16 block, so the device gathers 128 of them
    (one per neighbor n) with a single indirect DMA per (h, w-block).
  * matmul1 contracts n (=128, the partition dim) with the per-h psi basis
    vT [128, K]; 4 latitudes run concurrently in separate 32-column
    tile_position groups of the PE array.
  * z is evacuated from PSUM (f32->f16) and transposed via SBUF->SBUF DMA
    into a (k,c)-partition layout z' [K*CG, NG, Wout].
  * matmul2 contracts (c,k) in NG accumulating chunks against the packed
    weight w2 [K*CG, NG, Cout] to produce y[h] in PSUM, evacuated f32.
"""

import math
from contextlib import ExitStack
from dataclasses import dataclass

import numpy as np


# ---------------------------------------------------------------- dims

@dataclass(frozen=True)
class Dims:
    Cin: int = 73
    Hin: int = 721
    Win: int = 1440
    Cout: int = 256
    K: int = 9
    Hout: int = 361
    N: int = 128
    pscale: int = 2
    ncores: int = 8
    GH: int = 4     # latitudes processed concurrently (PE col-tile groups)
    WB: int = 60    # w-block size for the gather/matmul1 pipeline
    XB: int = 2     # gather buffer depth per latitude tag
    NHALF: int = 4  # z4 is staged in NHALF w-spans, transposed per span
    # "full" | "fakegather" | pipeline truncations for perf ablation:
    # "gather" (gathers+y only), "mm1", "evac", "tr" (everything but mm2)
    mode: str = "full"
    REP: int = 1    # repeat the whole body (device-side timing ablation)

    @property
    def stage(self):
        order = {"gather": 1, "mm1": 2, "evac": 3, "tr": 4,
                 "full": 5, "fakegather": 5}
        return order[self.mode]

    @property
    def Wout(self):
        return self.Win // self.pscale

    @property
    def WPAD(self):
        return 2 * self.Wout - 1

    @property
    def TOT(self):
        return 2 * self.Hin * self.WPAD * self.Cin

    @property
    def NWB(self):
        assert self.Wout % self.WB == 0
        return self.Wout // self.WB

    @property
    def WBH(self):  # w-blocks per z4 half-span
        assert self.NWB % self.NHALF == 0
        return self.NWB // self.NHALF

    @property
    def HWB(self):  # w width of a z4 span
        return self.WBH * self.WB

    @property
    def HB(self):  # padded per-core latitude slots
        per = math.ceil(self.Hout / self.ncores)
        return math.ceil(per / self.GH) * self.GH

    @property
    def NGRP(self):
        return self.HB // self.GH

    @property
    def KP(self):  # psi k-dim padded to a full PE column-tile group
        return 32

    @property
    def CG(self):  # channels per (c,k) partition group of z'
        return min(128 // self.K, self.Cin)

    @property
    def P2(self):
        return self.K * self.CG

    @property
    def NG(self):
        return math.ceil(self.Cin / self.CG)

    @property
    def CREM(self):  # channels in last group
        return self.Cin - self.CG * (self.NG - 1)

    @property
    def CC(self):  # channel chunk for matmul1 psum (<=512 f32 per bank)
        return max(1, min(512 // self.WB, self.Cin))

    @property
    def NCC(self):
        return math.ceil(self.Cin / self.CC)

    @property
    def OH(self):  # output-channel halves
        return math.ceil(self.Cout / 128)

    @property
    def OHW(self):
        return min(self.Cout, 128)

    @property
    def NWH(self):  # w-halves for matmul2 (<=512 f32 psum)
        return math.ceil(self.Wout / 512)

    @property
    def WHS(self):
        assert self.Wout % self.NWH == 0
        return self.Wout // self.NWH

    def check(self):
        assert self.N <= 128
        assert self.K <= 32
        assert 32 * (self.GH - 1) + self.K <= 128
        assert self.CC * self.WB <= 512
        assert self.WHS <= 512
        assert self.Win == 2 * self.Wout


# ---------------------------------------------------------------- device program

def build_nc(d: Dims):
    import concourse.bacc as bacc
    import concourse.bass as bass
    import concourse.tile as tile
    from concourse import mybir

    F16 = mybir.dt.float16
    F32 = mybir.dt.float32
    I32 = mybir.dt.int32

    d.check()
    nc = bacc.Bacc("TRN2", target_bir_lowering=False, debug=False,
                   num_devices=d.ncores)

    xpad = nc.declare_dram_parameter(
        "xpad", [2 * d.Hin, d.WPAD * d.Cin], F16, isOutput=False)
    gidx = nc.declare_dram_parameter("gidx", [d.N, d.HB], I32, isOutput=False)
    vt = nc.declare_dram_parameter("vt", [d.N, d.HB * d.KP], F16,
                                   isOutput=False)
    w2 = nc.declare_dram_parameter("w2", [d.P2, d.NG * d.Cout], F16,
                                   isOutput=False)
    y = nc.declare_dram_parameter("y", [d.HB, d.Cout, d.Wout], F32,
                                  isOutput=True)

    with tile.TileContext(nc) as tc, ExitStack() as ctx:
        const = ctx.enter_context(tc.tile_pool(name="const", bufs=1))
        xgp = ctx.enter_context(tc.tile_pool(name="xgp", bufs=2))
        z4p = ctx.enter_context(tc.tile_pool(name="z4p", bufs=2))
        zpsp = ctx.enter_context(tc.tile_pool(name="zpsp", bufs=4, space="PSUM"))
        zprp = ctx.enter_context(tc.tile_pool(name="zprp", bufs=1))
        ypsp = ctx.enter_context(tc.tile_pool(name="ypsp", bufs=2, space="PSUM"))
        ysbp = ctx.enter_context(tc.tile_pool(name="ysbp", bufs=3))

        gidx_sb = const.tile([d.N, d.HB], I32, name="gidx_sb")
        nc.sync.dma_start(out=gidx_sb[:], in_=gidx[:])
        vt_sb = const.tile([d.N, d.HB * d.KP], F16, name="vt_sb")
        nc.sync.dma_start(out=vt_sb[:], in_=vt[:])
        w2_sb = const.tile([d.P2, d.NG * d.Cout], F16, name="w2_sb")
        nc.sync.dma_start(out=w2_sb[:], in_=w2[:])

        vt_v = vt_sb.rearrange("n (h k) -> n h k", k=d.KP)
        w2_v = w2_sb.rearrange("p (g o) -> p g o", g=d.NG)

        # persistent z' tiles (one per concurrent latitude slot).
        # Group g occupies partitions [0, K*CGg) of free-column block g,
        # laid out p = k*CGg + co (per-group pitch keeps partitions
        # contiguous even for the short last group).
        zpr = [zprp.tile([d.P2, d.NG * d.Wout], F16, name=f"zpr_{j}",
                         tag=f"zpr_{j}") for j in range(d.GH)]

        for grp in range(d.NGRP * d.REP):
            grp = grp % d.NGRP
            hs = [grp * d.GH + j for j in range(d.GH)]
            for wb in range(d.NWB):
                half, wl = divmod(wb, d.WBH)
                xg = []
                for j in range(d.GH):
                    xgt = xgp.tile([d.N, d.WB * d.Cin], F16, name=f"xg{j}",
                                   tag=f"xg{j}", bufs=d.XB)
                    if d.mode == "fakegather":
                        r0 = ((grp * d.NWB + wb) * d.GH + j) % \
                            (2 * d.Hin - d.N)
                        nc.sync.dma_start(
                            out=xgt[:],
                            in_=xpad[r0:r0 + d.N,
                                     wb * d.WB * d.Cin:(wb + 1) * d.WB * d.Cin])
                    else:
                        nc.gpsimd.indirect_dma_start(
                            out=xgt[:],
                            out_offset=None,
                            in_=xpad[:],
                            in_offset=bass.IndirectOffsetOnAxis(
                                ap=gidx_sb[:, hs[j]:hs[j] + 1], axis=1),
                            element_offset=wb * d.WB * d.Cin,
                        )
                    xg.append(xgt)

                if wl == 0:
                    # only 2 spans alive at once: the one being written and
                    # the previous one being transposed out
                    z4 = z4p.tile([128, d.Cin * d.HWB], F16,
                                  name=f"z4{half % 2}",
                                  tag=f"z4{half % 2}", bufs=1)
                    z4_v = z4.rearrange("p (c w) -> p c w", c=d.Cin)
                if d.stage < 2:
                    continue
                for cc in range(d.NCC):
                    c0 = cc * d.CC
                    cw = min(d.CC, d.Cin - c0)
                    zps = zpsp.tile([128, d.CC * d.WB], F32, name="zps")
                    for j in range(d.GH):
                        rhs = xg[j].rearrange("n (w c) -> n c w",
                                              c=d.Cin)[:, c0:c0 + cw, :]
                        nc.tensor.matmul(
                            out=zps[32 * j:32 * (j + 1), :cw * d.WB],
                            lhsT=vt_v[:, hs[j], :],
                            rhs=rhs,
                            start=True, stop=True,
                            tile_position=(0, 32 * j),
                        )
                    if d.stage < 3:
                        continue
                    nc.vector.tensor_copy(
                        out=z4_v[:32 * d.GH, c0:c0 + cw,
                                 wl * d.WB:(wl + 1) * d.WB],
                        in_=zps.rearrange("p (c w) -> p c w",
                                          c=d.CC)[:32 * d.GH, :cw, :],
                    )

                if d.stage < 4 or wl != d.WBH - 1:
                    continue
                # span complete: long-run transposes (HWB-wide w runs)
                for j in range(d.GH):
                    zv = zpr[j].rearrange("p (g w) -> p g w", g=d.NG)
                    for g in range(d.NG):
                        cgg = d.CG if g < d.NG - 1 else d.CREM
                        # dst partitions p = k*cgg+co iterate (k, co) in the
                        # same lexicographic order as the src AP dims.
                        nc.sync.dma_start(
                            out=zv[:d.K * cgg, g,
                                   half * d.HWB:(half + 1) * d.HWB],
                            in_=z4_v[32 * j:32 * j + d.K,
                                     g * d.CG:g * d.CG + cgg, :],
                        )

            if d.stage < 5:
                # keep the y-write volume, fed from a junk tile
                for j in range(d.GH):
                    for oh in range(d.OH):
                        o0 = oh * d.OHW
                        ow = min(d.OHW, d.Cout - o0)
                        ysbt = ysbp.tile([d.OHW, d.Wout], F32, name="ysbt")
                        nc.vector.memset(ysbt[:1, :1], 0.0)
                        nc.sync.dma_start(
                            out=y[hs[j], o0:o0 + ow, :], in_=ysbt[:ow, :])
                continue
            for j in range(d.GH):
                zj = zpr[j].rearrange("p (g w) -> p g w", g=d.NG)
                for oh in range(d.OH):
                    o0 = oh * d.OHW
                    ow = min(d.OHW, d.Cout - o0)
                    ysbt = ysbp.tile([d.OHW, d.Wout], F32, name="ysbt")
                    yps = [ypsp.tile([d.OHW, d.WHS], F32, name=f"yps{wh}",
                                     tag=f"yps{wh}")
                           for wh in range(d.NWH)]
                    for g in range(d.NG):
                        cgg = d.CG if g < d.NG - 1 else d.CREM
                        for wh in range(d.NWH):
                            nc.tensor.matmul(
                                out=yps[wh][:ow, :],
                                lhsT=w2_v[:d.K * cgg, g, o0:o0 + ow],
                                rhs=zj[:d.K * cgg, g,
                                       wh * d.WHS:(wh + 1) * d.WHS],
                                start=(g == 0), stop=(g == d.NG - 1),
                            )
                    for wh in range(d.NWH):
                        nc.vector.tensor_copy(
                            out=ysbt[:ow, wh * d.WHS:(wh + 1) * d.WHS],
                            in_=yps[wh][:ow, :])
                    nc.sync.dma_start(
                        out=y[hs[j], o0:o0 + ow, :],
                        in_=ysbt[:ow, :])

    nc.finalize()
    return nc


# ---------------------------------------------------------------- host side

def prep_xpad(x, d: Dims):
    xr = np.transpose(x[0], (1, 2, 0))  # [Hin, Win, Cin]
    xs = xr.reshape(d.Hin, d.Wout, d.pscale, d.Cin).transpose(2, 0, 1, 3)
    xpad = np.empty((2, d.Hin, d.WPAD, d.Cin), dtype=np.float16)
    xpad[:, :, :d.Wout] = xs
    xpad[:, :, d.Wout:] = xs[:, :, :d.WPAD - d.Wout]
    return np.ascontiguousarray(xpad).reshape(2 * d.Hin, d.WPAD * d.Cin)


def core_h_ranges(d: Dims):
    base, rem = divmod(d.Hout, d.ncores)
    counts = [base + (1 if p < rem else 0) for p in range(d.ncores)]
    offs = np.concatenate([[0], np.cumsum(counts)])
    return [(int(offs[p]), counts[p]) for p in range(d.ncores)]


def prep_core_tables(psi_vals, idx_hi, idx_wi, d: Dims, h0, cnt):
    hg = np.minimum(h0 + np.arange(d.HB), d.Hout - 1)  # pad with a valid h
    wi = idx_wi[hg]                      # [HB, N]
    par = wi % 2
    m = wi // 2
    r = idx_hi[hg]
    flat = ((par.astype(np.int64) * d.Hin + r) * d.WPAD + m) * d.Cin
    assert flat.max() + d.Wout * d.Cin <= d.TOT
    gidx = flat.astype(np.int32).T.copy()           # [N, HB]
    vt = np.zeros((d.N, d.HB, d.KP), dtype=np.float16)
    vt[:, :, :d.K] = psi_vals[:, hg, :].transpose(2, 1, 0)
    return gidx, vt.reshape(d.N, d.HB * d.KP)


def prep_w2(weight, d: Dims):
    w = weight.transpose(1, 2, 0)  # [Cin, K, Cout]
    w2 = np.zeros((d.P2, d.NG, d.Cout), dtype=np.float16)
    for g in range(d.NG):
        cs = g * d.CG
        cgg = d.CG if g < d.NG - 1 else d.CREM
        # rows p = k*cgg + co
        blk = w[cs:cs + cgg].transpose(1, 0, 2).reshape(d.K * cgg, d.Cout)
        w2[:d.K * cgg, g] = blk
    return np.ascontiguousarray(w2.reshape(d.P2, d.NG * d.Cout))


_NC_CACHE = {}


def _get_nc(d: Dims):
    if d not in _NC_CACHE:
        _NC_CACHE[d] = build_nc(d)
    return _NC_CACHE[d]


def make_in_maps(x, weight, psi_vals, idx_hi, idx_wi, d: Dims):
    xpad = prep_xpad(x, d)
    w2 = prep_w2(weight, d)
    in_maps = []
    for h0, cnt in core_h_ranges(d):
        gidx, vt = prep_core_tables(psi_vals, idx_hi, idx_wi, d, h0, cnt)
        in_maps.append({"xpad": xpad, "gidx": gidx, "vt": vt, "w2": w2})
    return in_maps


def assemble_y(per_core_y, d: Dims):
    parts = [per_core_y[p][:cnt] for p, (h0, cnt) in enumerate(core_h_ranges(d))]
    yh = np.concatenate(parts, axis=0)          # [Hout, Cout, Wout]
    return yh.transpose(1, 0, 2)[None]          # [1, Cout, Hout, Wout]


def kernel(x, weight, psi_vals, idx_hi, idx_wi):
    from concourse.bass_utils import run_bass_kernel_spmd

    d = Dims()
    nc = _get_nc(d)
    in_maps = make_in_maps(x, weight, psi_vals, idx_hi, idx_wi, d)
    res = run_bass_kernel_spmd(nc, in_maps, list(range(d.ncores)))
    ys = [res.results[p]["y"] for p in range(d.ncores)]
    return assemble_y(ys, d).astype(x.dtype)
